# revision 2
# baseline (speedup 1.0000x reference)
"""Trainium2 Bass kernel for CatFeaturesItemNet (EmbeddingBag sum, segment_reduce).

Strategy (data-parallel over items, table replicated — per sharding hint):
  * 8 cores, 8192 items each.
  * Host-side index prep (the "CSR side"): gather per-item feature ids
    (offsets/lengths/emb_bag_inputs are int32 index structures), sort each
    core's items by bag length (descending), build per-group gather streams.
  * Device does all f32 payload movement + reduction:
      - weight table viewed as [25000, 512] f32 (2KB blocks of 4 rows) so
        block ids fit dma_gather's int16 index limit (id>>2 < 25000).
      - per group of 128 items: one dma_gather pulls L lanes/item
        (column-major: lane t of item p lands at dest[p, t, :512]).
      - DVE: in-place multiply by uploaded phase-mask (selects the right
        128-f32 row of each 2KB block, zeroes padding lanes), then
        tensor_reduce over (lane, phase) -> [128 items, 128] f32.
      - store per group; host unpermutes rows back to original item order.
  * Lane budget per group is a static schedule derived from binomial bounds
    on sorted uniform{1..16} lengths — program shape is input-independent;
    only tensor contents depend on the inputs.
"""

import numpy as np
from contextlib import ExitStack

N_CORES = 8
BATCH = 65536
BL = BATCH // N_CORES          # items per core
L_MAX = 16
D = 128
V = 100000                     # weight rows
NBLK = V // 4                  # 2KB blocks (4 rows each)
ELEM = 512                     # f32 per gathered block (2KB)
GROUPS = BL // 128             # 64 groups of 128 items per core
SAFETY_SIGMA = 10.0


def _static_lane_schedule(n_items=BL, groups=GROUPS):
    """L_hat[g]: static upper bound on the max bag length within group g of
    128 items after sorting lengths (uniform{1..16}) in descending order.
    Group g's max length exceeds L only if count(len >= L+1) > 128*g;
    count(len >= k) ~ Binomial(n, (17-k)/16)."""
    sched = []
    for g in range(groups):
        lhat = L_MAX
        for L in range(1, L_MAX + 1):
            p = (L_MAX - L) / 16.0  # P(len >= L+1) for len ~ uniform{1..16}
            mean = n_items * p
            sigma = np.sqrt(n_items * p * (1 - p))
            if mean + SAFETY_SIGMA * sigma <= g * 128:
                lhat = L
                break
        sched.append(lhat)
    return sched


L_SCHED = _static_lane_schedule()


def _build_bass():
    import concourse.bass as bass
    import concourse.bacc as bacc
    import concourse.tile as tile
    from concourse import mybir
    from concourse.library_config import mlp

    idx_cols = sum(128 * L // 16 for L in L_SCHED)        # int16 cols
    mask_cols = sum(L * 4 for L in L_SCHED)               # f32 cols

    nc = bacc.Bacc("TRN2", target_bir_lowering=False, debug=False,
                   num_devices=N_CORES)
    weight = nc.declare_dram_parameter("weight", [V, D], mybir.dt.float32,
                                       isOutput=False)
    idx_in = nc.declare_dram_parameter("idx", [128, idx_cols], mybir.dt.int16,
                                       isOutput=False)
    mask_in = nc.declare_dram_parameter("mask", [128, mask_cols],
                                        mybir.dt.float32, isOutput=False)
    out = nc.declare_dram_parameter("out", [BL, D], mybir.dt.float32,
                                    isOutput=True)

    wblk = weight.rearrange("(a b) d -> a (b d)", b=4)    # [25000, 512]
    out_g = out.rearrange("(g p) d -> g p d", p=128)      # [64, 128, 128]

    with tile.TileContext(nc) as tc:
        with ExitStack() as ctx:
            cons = ctx.enter_context(tc.tile_pool(name="cons", bufs=1))
            gp = ctx.enter_context(tc.tile_pool(name="g", bufs=3))
            op = ctx.enter_context(tc.tile_pool(name="o", bufs=3))

            nc.gpsimd.load_library(mlp)
            idx_t = cons.tile([128, idx_cols], mybir.dt.int16)
            nc.gpsimd.dma_start(out=idx_t[:], in_=idx_in[:, :])
            mask_t = cons.tile([128, mask_cols], mybir.dt.float32)
            nc.sync.dma_start(out=mask_t[:], in_=mask_in[:, :])

            ic = 0   # running idx col offset
            mc = 0   # running mask col offset
            for g, L in enumerate(L_SCHED):
                ni = 128 * L
                gt = gp.tile([128, L_MAX * ELEM], mybir.dt.float32, tag="g")
                gv = gt[:, :L * ELEM]
                nc.gpsimd.dma_gather(
                    out_ap=gv.rearrange("p (c e) -> p c e", e=ELEM),
                    in_ap=wblk[:, :],
                    idxs_ap=idx_t[:, ic:ic + ni // 16],
                    num_idxs=ni,
                    num_idxs_reg=ni,
                    elem_size=ELEM,
                    single_packet=False,
                )
                # select phase row + zero pad lanes:  G *= M (broadcast over d)
                g4 = gv.rearrange("p (t q d) -> p t q d", q=4, d=D)
                m4 = mask_t[:, mc:mc + L * 4].rearrange(
                    "p (t q) -> p t q", q=4).to_broadcast([128, L, 4, D])
                nc.vector.tensor_tensor(out=g4, in0=g4, in1=m4,
                                        op=mybir.AluOpType.mult)
                # sum over (t, q): innermost two dims of [p][d][t][q]
                o_t = op.tile([128, D], mybir.dt.float32, tag="o")
                rin = gv.rearrange("p (t q d) -> p d t q", q=4, d=D)
                nc.vector.tensor_reduce(out=o_t[:], in_=rin,
                                        axis=mybir.AxisListType.XY,
                                        op=mybir.AluOpType.add)
                nc.sync.dma_start(out=out_g[g], in_=o_t[:])
                ic += ni // 16
                mc += L * 4
    nc.compile()
    return nc, idx_cols, mask_cols


def _host_prep(core_items, emb_bag_inputs, offsets, input_lengths,
               idx_cols, mask_cols):
    """Build per-core idx/mask tensors + the inverse permutation."""
    it = core_items.astype(np.int64)
    off = offsets[it].astype(np.int64)
    ln = input_lengths[it].astype(np.int64)
    ids = emb_bag_inputs[off[:, None] + np.arange(L_MAX)[None, :]].astype(np.int64)

    order = np.argsort(-ln, kind="stable")      # items sorted by len desc
    ln_s = ln[order]
    ids_s = ids[order]

    idx_arr = np.zeros((128, idx_cols), dtype=np.int16)
    mask_arr = np.zeros((128, mask_cols), dtype=np.float32)
    ic = 0
    mc = 0
    for g, L in enumerate(L_SCHED):
        sl = slice(g * 128, (g + 1) * 128)
        ln_g = ln_s[sl]                          # [128]
        if ln_g.max(initial=0) > L:
            raise RuntimeError(
                f"static lane schedule violated in group {g}: "
                f"max len {ln_g.max()} > {L}")
        ids_g = ids_s[sl]                        # [128, 16]
        lanes = np.minimum(np.arange(L)[None, :], ln_g[:, None] - 1)  # pad->dup lane0.. actually dup of clamped lane
        lane_ids = np.take_along_axis(ids_g, lanes, axis=1)  # [128, L]
        blk = (lane_ids >> 2).astype(np.int16)               # [128, L]
        ph = (lane_ids & 3).astype(np.int64)                 # [128, L]
        valid = (np.arange(L)[None, :] < ln_g[:, None])      # [128, L]

        # column-major stream: s = t*128 + p
        stream = blk.T.reshape(-1)                           # [128*L]
        ni = 128 * L
        wrapped = stream.reshape(ni // 16, 16).T             # [16, ni/16]
        idx_arr[:, ic:ic + ni // 16] = np.tile(wrapped, (8, 1))

        m = np.zeros((128, L, 4), dtype=np.float32)
        np.put_along_axis(m, ph[:, :, None], 1.0, axis=2)
        m *= valid[:, :, None]
        mask_arr[:, mc:mc + L * 4] = m.reshape(128, L * 4)
        ic += ni // 16
        mc += L * 4

    inv = np.empty(BL, dtype=np.int64)
    inv[order] = np.arange(BL)                  # original j -> sorted row
    return idx_arr, mask_arr, inv


_CACHE = {}


def kernel(items, emb_bag_inputs, offsets, input_lengths, weight):
    from concourse.bass_utils import run_bass_kernel_spmd

    if "nc" not in _CACHE:
        _CACHE["nc"], _CACHE["icols"], _CACHE["mcols"] = _build_bass()
    nc = _CACHE["nc"]
    icols, mcols = _CACHE["icols"], _CACHE["mcols"]

    weight_f32 = np.ascontiguousarray(weight, dtype=np.float32)
    in_maps = []
    invs = []
    for c in range(N_CORES):
        idx_arr, mask_arr, inv = _host_prep(
            np.asarray(items[c * BL:(c + 1) * BL]),
            np.asarray(emb_bag_inputs), np.asarray(offsets),
            np.asarray(input_lengths), icols, mcols)
        in_maps.append({"weight": weight_f32, "idx": idx_arr,
                        "mask": mask_arr})
        invs.append(inv)

    res = run_bass_kernel_spmd(nc, in_maps, list(range(N_CORES)))
    outs = []
    for c in range(N_CORES):
        dev = res.results[c]["out"]            # [BL, D] in sorted order
        outs.append(dev[invs[c]])
    return np.concatenate(outs, axis=0).astype(np.float32)


# revision 3
# speedup vs baseline: 873.3464x; 873.3464x over previous
"""Trainium2 Bass kernel for CatFeaturesItemNet (EmbeddingBag sum, segment_reduce).

Strategy (data-parallel over items, table replicated — per sharding hint):
  * 8 cores, 8192 items each.
  * Host-side index prep (the "CSR side"): gather per-item feature ids
    (offsets/lengths/emb_bag_inputs are int32 index structures), sort each
    core's items by bag length (descending), build per-group gather streams.
  * Device does all f32 payload movement + reduction:
      - weight table viewed as [25000, 512] f32 (2KB blocks of 4 rows) so
        block ids fit dma_gather's int16 index limit (id>>2 < 25000).
      - per group of 128 items: one dma_gather pulls L lanes/item
        (column-major: lane t of item p lands at dest[p, t, :512]).
      - DVE: in-place multiply by uploaded phase-mask (selects the right
        128-f32 row of each 2KB block, zeroes padding lanes), then
        tensor_reduce over (lane, phase) -> [128 items, 128] f32.
      - store per group; host unpermutes rows back to original item order.
  * Lane budget per group is a static schedule derived from binomial bounds
    on sorted uniform{1..16} lengths — program shape is input-independent;
    only tensor contents depend on the inputs.
"""

import numpy as np
from contextlib import ExitStack

N_CORES = 8
BATCH = 65536
BL = BATCH // N_CORES          # items per core
L_MAX = 16
D = 128
V = 100000                     # weight rows
NBLK = V // 4                  # 2KB blocks (4 rows each)
ELEM = 512                     # f32 per gathered block (2KB)
GROUPS = BL // 128             # 64 groups of 128 items per core
SAFETY_SIGMA = 10.0


def _static_lane_schedule(n_items=BL, groups=GROUPS):
    """L_hat[g]: static upper bound on the max bag length within group g of
    128 items after sorting lengths (uniform{1..16}) in descending order.
    Group g's max length exceeds L only if count(len >= L+1) > 128*g;
    count(len >= k) ~ Binomial(n, (17-k)/16)."""
    sched = []
    for g in range(groups):
        lhat = L_MAX
        for L in range(1, L_MAX + 1):
            p = (L_MAX - L) / 16.0  # P(len >= L+1) for len ~ uniform{1..16}
            mean = n_items * p
            sigma = np.sqrt(n_items * p * (1 - p))
            if mean + SAFETY_SIGMA * sigma <= g * 128:
                lhat = L
                break
        sched.append(lhat)
    return sched


L_SCHED = _static_lane_schedule()


def _build_bass():
    import concourse.bass as bass
    import concourse.bacc as bacc
    import concourse.tile as tile
    from concourse import mybir
    from concourse.library_config import mlp

    idx_cols = sum(128 * L // 16 for L in L_SCHED)        # int16 cols
    mask_cols = sum(L * 4 for L in L_SCHED)               # f32 cols

    nc = bacc.Bacc("TRN2", target_bir_lowering=False, debug=False,
                   num_devices=N_CORES)
    weight = nc.declare_dram_parameter("weight", [V, D], mybir.dt.float32,
                                       isOutput=False)
    idx_in = nc.declare_dram_parameter("idx", [128, idx_cols], mybir.dt.int16,
                                       isOutput=False)
    mask_in = nc.declare_dram_parameter("mask", [128, mask_cols],
                                        mybir.dt.float32, isOutput=False)
    out = nc.declare_dram_parameter("out", [BL, D], mybir.dt.float32,
                                    isOutput=True)

    wblk = weight.rearrange("(a b) d -> a (b d)", b=4)    # [25000, 512]
    out_g = out.rearrange("(g p) d -> g p d", p=128)      # [64, 128, 128]

    with tile.TileContext(nc) as tc:
        with ExitStack() as ctx:
            cons = ctx.enter_context(tc.tile_pool(name="cons", bufs=1))
            gp = ctx.enter_context(tc.tile_pool(name="g", bufs=4))
            op = ctx.enter_context(tc.tile_pool(name="o", bufs=3))

            nc.gpsimd.load_library(mlp)
            idx_t = cons.tile([128, idx_cols], mybir.dt.int16)
            nc.gpsimd.dma_start(out=idx_t[:], in_=idx_in[:, :])
            mask_t = cons.tile([128, mask_cols], mybir.dt.float32)
            nc.sync.dma_start(out=mask_t[:], in_=mask_in[:, :])

            ic = 0   # running idx col offset
            mc = 0   # running mask col offset
            for g, L in enumerate(L_SCHED):
                ni = 128 * L
                gt = gp.tile([128, L_MAX * ELEM], mybir.dt.float32, tag="g")
                gv = gt[:, :L * ELEM]
                nc.gpsimd.dma_gather(
                    out_ap=gv.rearrange("p (c e) -> p c e", e=ELEM),
                    in_ap=wblk[:, :],
                    idxs_ap=idx_t[:, ic:ic + ni // 16],
                    num_idxs=ni,
                    num_idxs_reg=ni,
                    elem_size=ELEM,
                    single_packet=False,
                )
                # select phase row + zero pad lanes:  G *= M (broadcast over d)
                g4 = gv.rearrange("p (t q d) -> p t q d", q=4, d=D)
                m4 = mask_t[:, mc:mc + L * 4].rearrange(
                    "p (t q) -> p t q", q=4).to_broadcast([128, L, 4, D])
                nc.vector.tensor_tensor(out=g4, in0=g4, in1=m4,
                                        op=mybir.AluOpType.mult)
                # sum over (t, q): innermost two dims of [p][d][t][q]
                o_t = op.tile([128, D], mybir.dt.float32, tag="o")
                rin = gv.rearrange("p (t q d) -> p d t q", q=4, d=D)
                nc.vector.tensor_reduce(out=o_t[:], in_=rin,
                                        axis=mybir.AxisListType.XY,
                                        op=mybir.AluOpType.add)
                nc.sync.dma_start(out=out_g[g], in_=o_t[:])
                ic += ni // 16
                mc += L * 4
    nc.compile()
    return nc, idx_cols, mask_cols


def _host_prep(core_items, emb_bag_inputs, offsets, input_lengths,
               idx_cols, mask_cols):
    """Build per-core idx/mask tensors + the inverse permutation."""
    it = core_items.astype(np.int64)
    off = offsets[it].astype(np.int64)
    ln = input_lengths[it].astype(np.int64)
    ids = emb_bag_inputs[off[:, None] + np.arange(L_MAX)[None, :]].astype(np.int64)

    order = np.argsort(-ln, kind="stable")      # items sorted by len desc
    ln_s = ln[order]
    ids_s = ids[order]

    idx_arr = np.zeros((128, idx_cols), dtype=np.int16)
    mask_arr = np.zeros((128, mask_cols), dtype=np.float32)
    ic = 0
    mc = 0
    for g, L in enumerate(L_SCHED):
        sl = slice(g * 128, (g + 1) * 128)
        ln_g = ln_s[sl]                          # [128]
        if ln_g.max(initial=0) > L:
            raise RuntimeError(
                f"static lane schedule violated in group {g}: "
                f"max len {ln_g.max()} > {L}")
        ids_g = ids_s[sl]                        # [128, 16]
        lanes = np.minimum(np.arange(L)[None, :], ln_g[:, None] - 1)  # pad->dup lane0.. actually dup of clamped lane
        lane_ids = np.take_along_axis(ids_g, lanes, axis=1)  # [128, L]
        blk = (lane_ids >> 2).astype(np.int16)               # [128, L]
        ph = (lane_ids & 3).astype(np.int64)                 # [128, L]
        valid = (np.arange(L)[None, :] < ln_g[:, None])      # [128, L]

        # column-major stream: s = t*128 + p
        stream = blk.T.reshape(-1)                           # [128*L]
        ni = 128 * L
        wrapped = stream.reshape(ni // 16, 16).T             # [16, ni/16]
        idx_arr[:, ic:ic + ni // 16] = np.tile(wrapped, (8, 1))

        m = np.zeros((128, L, 4), dtype=np.float32)
        np.put_along_axis(m, ph[:, :, None], 1.0, axis=2)
        m *= valid[:, :, None]
        mask_arr[:, mc:mc + L * 4] = m.reshape(128, L * 4)
        ic += ni // 16
        mc += L * 4

    inv = np.empty(BL, dtype=np.int64)
    inv[order] = np.arange(BL)                  # original j -> sorted row
    return idx_arr, mask_arr, inv


_CACHE = {}


def kernel(items, emb_bag_inputs, offsets, input_lengths, weight):
    from concourse.bass_utils import run_bass_kernel_spmd

    if "nc" not in _CACHE:
        _CACHE["nc"], _CACHE["icols"], _CACHE["mcols"] = _build_bass()
    nc = _CACHE["nc"]
    icols, mcols = _CACHE["icols"], _CACHE["mcols"]

    weight_f32 = np.ascontiguousarray(weight, dtype=np.float32)
    in_maps = []
    invs = []
    for c in range(N_CORES):
        idx_arr, mask_arr, inv = _host_prep(
            np.asarray(items[c * BL:(c + 1) * BL]),
            np.asarray(emb_bag_inputs), np.asarray(offsets),
            np.asarray(input_lengths), icols, mcols)
        in_maps.append({"weight": weight_f32, "idx": idx_arr,
                        "mask": mask_arr})
        invs.append(inv)

    res = run_bass_kernel_spmd(nc, in_maps, list(range(N_CORES)))
    outs = []
    for c in range(N_CORES):
        dev = res.results[c]["out"]            # [BL, D] in sorted order
        outs.append(dev[invs[c]])
    return np.concatenate(outs, axis=0).astype(np.float32)
